# revision 15
# baseline (speedup 1.0000x reference)
"""Trainium2 Bass kernel for nn_CRELayer (LayerNorm -> gated push/pull cumsum -> decayed output proj -> residual).

Contract: kernel(**inputs) takes the FULL unsharded inputs (numpy) and returns
the FULL (4, 8192, 1024) float32 output.

Sharding: 8 cores; core c handles (batch b=c//2, T-half h=c%2) -> a (4096, 1024)
row-slice. The time-cumsum crosses the half boundary, so each core computes
per-channel column sums of u = tanh(z/2)*v and a pairwise in-kernel AllReduce
hands the half-0 totals to the half-1 core as the scan's initial state
(gated to zero on half-0 cores).

Math notes:
  - LayerNorm gamma/beta are folded into Wg/Wv + biases on the host.
  - cumsum(g*v) - cumsum((1-g)*v) = cumsum((2g-1)*v) and 2*sigmoid(z)-1 = tanh(z/2),
    so the gate+value combine is a single ACT tanh pass.
  - positional decay is applied per-partition AFTER the Wo matmul:
    (cumsum*decay) @ Wo.T == decay[t] * (cumsum @ Wo.T).
"""

import sys
import types

for _p in ("/opt/trn_rl_repo",):
    if _p not in sys.path:
        sys.path.insert(0, _p)

import ml_dtypes
import numpy as np

# ---------------------------------------------------------------------------
# Environment shims (axon image quirks), applied once at import.
# ---------------------------------------------------------------------------


def _install_axon_hooks_shim():
    """The image's antenv package lacks axon_hooks; provide it so
    run_bass_kernel_spmd(trace=True) can collect NTFF profiles."""
    if "antenv.axon_hooks" in sys.modules:
        return
    hooks = types.ModuleType("antenv.axon_hooks")
    hooks._hook = None

    def set_axon_ntff_profile_hook(h):
        hooks._hook = h

    def get_axon_ntff_profile_hook():
        return hooks._hook

    hooks.set_axon_ntff_profile_hook = set_axon_ntff_profile_hook
    hooks.get_axon_ntff_profile_hook = get_axon_ntff_profile_hook
    sys.modules["antenv.axon_hooks"] = hooks
    try:
        from trn_agent_boot.trn_boot import _ntff_profile_via_ctypes

        set_axon_ntff_profile_hook(
            _ntff_profile_via_ctypes("/opt/axon/libaxon_pjrt.so")
        )
    except Exception:
        pass


_install_axon_hooks_shim()

import bass_rust
import concourse.bass as bass
import concourse.tile as tile
from concourse import mybir
from concourse.bass_utils import run_bass_kernel_spmd
from concourse.vector_clock import ScopedClock


def _patch_tile_drain():
    """This walrus build only accepts one sem wait on sync-engine
    instructions; the stock TileContext exit drain carries one wait per live
    proc. Re-emit those waits as individual sync nops (same semantics)."""
    if getattr(tile.TileContext, "_drain_patch_installed", False):
        return

    def _drain_and_barrier(self, tick_clock, wait_clock):
        nc = self.nc
        probe = nc.sync.nop()
        wait_clock.add_sem_waits(
            probe.ins, ScopedClock({None: tick_clock.global_clock})
        )
        si = probe.ins.sync_info
        waits = list(si.on_wait) if si is not None and si.on_wait else []
        if len(waits) > 1:
            si.on_wait = waits[:1]
            for i in range(1, len(waits)):
                nop2 = nc.sync.nop()
                nop2.ins.sync_info = bass_rust.SyncInfo(
                    on_wait=waits[i : i + 1], on_update=[]
                )
        nc.sync.drain()
        nc.all_engine_barrier()
        assert self.sems is not None
        popped = nc._tile_sem_poison_stack.pop()
        assert popped is self._sem_poison
        nc.clear_and_free_semaphores(list(self.sems.allocated().values()))
        nc.all_engine_barrier()

    tile.TileContext._drain_and_barrier = _drain_and_barrier
    tile.TileContext._drain_patch_installed = True


_patch_tile_drain()

# ---------------------------------------------------------------------------
# Problem constants (hardcoded per contract).
# ---------------------------------------------------------------------------

B, T, D = 4, 8192, 1024
EPS = 1e-5
N_CORES = 8
TL = T // 2          # rows per core
NB = 8               # t-blocks per core
TB = TL // NB        # 512 rows per block
Nb_T = TB // 128     # 4 t-tiles per block
NCH = D // 128       # 8 chunks of 128 channels

F32 = mybir.dt.float32
F32R = mybir.dt.float32r
BF16 = mybir.dt.bfloat16
U32 = mybir.dt.uint32
AF = mybir.ActivationFunctionType
ALU = mybir.AluOpType

_PROGRAM_CACHE = {}

_WAIT_CAP = 1  # this walrus rejects TPB instructions with >1 sync wait


def _split_excess_waits(nc, cap=_WAIT_CAP):
    """Hoist excess sem waits onto same-engine nops placed just before the
    owning instruction. Engine queues execute in order, so a preceding nop
    carrying a wait gates the instruction identically."""
    n_split = 0
    for bb in nc.m.functions[0].blocks:
        insts = bb.instructions
        new_insts = []
        for inst in insts:
            si = inst.sync_info
            waits = list(si.on_wait) if si is not None and si.on_wait else []
            if len(waits) > cap:
                extra = waits[:-cap]
                for w in extra:
                    nop = mybir.InstNoOp(
                        name=f"waitsplit-{n_split}", ins=[], outs=[]
                    )
                    n_split += 1
                    nop.engine = inst.engine
                    nop.sync_info = bass_rust.SyncInfo(
                        on_wait=[w], on_update=[]
                    )
                    new_insts.append(nop)
                si.on_wait = waits[-cap:]
            new_insts.append(inst)
        if n_split:
            bb.instructions = new_insts
    return n_split


def _build_program(bo_nonzero: bool) -> bass.Bass:
    nc = bass.Bass(trn_type="TRN2")

    xs = nc.declare_dram_parameter("xs", [TL, D], F32, isOutput=False)
    wgT = nc.declare_dram_parameter("wgT", [128, NCH, D], BF16, isOutput=False)
    wvT = nc.declare_dram_parameter("wvT", [128, NCH, D], BF16, isOutput=False)
    woT = nc.declare_dram_parameter("woT", [128, NCH, D], BF16, isOutput=False)
    bgh = nc.declare_dram_parameter("bgh", [D], F32, isOutput=False)
    bvp = nc.declare_dram_parameter("bvp", [D], F32, isOutput=False)
    dec = nc.declare_dram_parameter("dec", [TL], F32, isOutput=False)
    gate = nc.declare_dram_parameter("gate", [128, 1], F32, isOutput=False)
    ident = nc.declare_dram_parameter("ident", [128, 128], BF16, isOutput=False)
    if bo_nonzero:
        bob = nc.declare_dram_parameter("bob", [D], F32, isOutput=False)
    out = nc.declare_dram_parameter("out", [TL, D], F32, isOutput=True)

    u_d = nc.dram_tensor("u_spill", [NB, NCH, 128, TB], BF16)
    cc_in = nc.dram_tensor("cc_in", [128, NCH], F32)
    cc_out = nc.dram_tensor("cc_out", [128, NCH], F32)
    wcorr_d = nc.dram_tensor("wcorr", [D], F32)

    with tile.TileContext(nc) as tc:
        with tc.tile_pool(name="const", bufs=1) as cpool:
            ident_sb = cpool.tile([128, 128], BF16)
            nc.sync.dma_start(ident_sb[:], ident[:])
            bgh_sb = cpool.tile([128, NCH], F32)
            nc.sync.dma_start(bgh_sb[:], bgh[:].rearrange("(j p) -> p j", p=128))
            bvp_sb = cpool.tile([128, NCH], F32)
            nc.sync.dma_start(bvp_sb[:], bvp[:].rearrange("(j p) -> p j", p=128))
            dec_sb = cpool.tile([128, TL // 128], F32)
            nc.sync.dma_start(dec_sb[:], dec[:].rearrange("(i p) -> p i", p=128))
            gate_sb = cpool.tile([128, 1], F32)
            nc.sync.dma_start(gate_sb[:], gate[:])
            if bo_nonzero:
                bo_sb = cpool.tile([128, D], F32)
                nc.sync.dma_start(
                    bo_sb[:], bob[:].unsqueeze(0).partition_broadcast(128)
                )
            magic_sb = cpool.tile([128, Nb_T], U32)
            nc.vector.memset(magic_sb[:], 0x5F3759DF)
            psums_sb = cpool.tile([128, NB * NCH], F32)
            carry_sb = cpool.tile([128, NCH], F32)
            lcarry_sb = cpool.tile([128, NCH], F32)
            nc.vector.memset(lcarry_sb[:], 0.0)
            carry_bf = cpool.tile([128, NCH], BF16)
            wc_sb = cpool.tile([1, D], F32)
            wcb_sb = cpool.tile([128, D], F32)
            wo_sb = cpool.tile([128, NCH, D], BF16)
            nc.scalar.dma_start(wo_sb[:], woT[:])
            colsum_sb = cpool.tile([128, NCH], F32)
            allred_sb = cpool.tile([128, NCH], F32)

            # ---------------- Phase A: LN -> transpose -> Wg/Wv -> u ----
            with tc.tile_pool(name="wA", bufs=1) as wpool, \
                 tc.tile_pool(name="xt", bufs=6) as xtp, \
                 tc.tile_pool(name="xb", bufs=6) as xbp, \
                 tc.tile_pool(name="yT", bufs=3) as yTp, \
                 tc.tile_pool(name="hv", bufs=6) as hvp, \
                 tc.tile_pool(name="ub", bufs=6) as up, \
                 tc.tile_pool(name="stat", bufs=10) as stp, \
                 tc.tile_pool(name="ptr", bufs=2, space="PSUM") as ptrp, \
                 tc.tile_pool(name="pg", bufs=3, space="PSUM") as pgp, \
                 tc.tile_pool(name="pv", bufs=3, space="PSUM") as pvp:

                wg_sb = wpool.tile([128, NCH, D], BF16)
                nc.scalar.dma_start(wg_sb[:], wgT[:])
                wv_sb = wpool.tile([128, NCH, D], BF16)
                nc.scalar.dma_start(wv_sb[:], wvT[:])

                for b in range(NB):
                    xts = []
                    mvs = stp.tile([128, Nb_T, 2], F32)
                    raw = []
                    for i in range(Nb_T):
                        r0 = (b * Nb_T + i) * 128
                        xt = xtp.tile([128, D], F32)
                        nc.sync.dma_start(xt[:], xs[r0 : r0 + 128, :])
                        st6 = stp.tile([128, 2, 6], F32)
                        for a in range(2):
                            nc.vector.bn_stats(
                                st6[:, a, :], xt[:, a * 512 : (a + 1) * 512]
                            )
                        nc.vector.bn_aggr(mvs[:, i, :], st6[:])
                        raw.append(xt)
                    # s = rsqrt(var+eps) for the whole block: magic + 3 Newton
                    w = stp.tile([128, Nb_T], F32)
                    nc.vector.tensor_scalar_add(w[:], mvs[:, :, 1], EPS)
                    ti = stp.tile([128, Nb_T], U32)
                    nc.vector.tensor_scalar(
                        ti[:], w[:].bitcast(U32), 1, None,
                        op0=ALU.logical_shift_right,
                    )
                    nc.vector.tensor_sub(ti[:], magic_sb[:], ti[:])
                    sv = ti[:].bitcast(F32)
                    t2 = stp.tile([128, Nb_T], F32)
                    for _ in range(3):
                        nc.vector.tensor_mul(t2[:], sv, sv)
                        nc.vector.tensor_mul(t2[:], t2[:], w[:])
                        nc.vector.tensor_scalar(
                            t2[:], t2[:], -0.5, 1.5, op0=ALU.mult, op1=ALU.add
                        )
                        nc.vector.tensor_mul(sv, sv, t2[:])
                    for i in range(Nb_T):
                        xb = xbp.tile([128, D], BF16)
                        nc.vector.tensor_scalar(
                            xb[:], raw[i][:], mvs[:, i, 0:1], sv[:, i : i + 1],
                            op0=ALU.subtract, op1=ALU.mult,
                        )
                        xts.append(xb)

                    yTt = yTp.tile([128, NCH, TB], BF16)
                    for k in range(NCH):
                        ptr = ptrp.tile([128, TB], BF16)
                        for i in range(Nb_T):
                            nc.tensor.transpose(
                                ptr[:, i * 128 : (i + 1) * 128],
                                xts[i][:, k * 128 : (k + 1) * 128],
                                ident_sb[:],
                            )
                        nc.scalar.copy(yTt[:, k, :], ptr[:])

                    for j in range(NCH):
                        pg = pgp.tile([128, TB], F32)
                        for k in range(NCH):
                            nc.tensor.matmul(
                                pg[:],
                                wg_sb[:, k, j * 128 : (j + 1) * 128],
                                yTt[:, k, :],
                                start=(k == 0),
                                stop=(k == NCH - 1),
                            )
                        h = hvp.tile([128, TB], BF16)
                        nc.scalar.activation(
                            h[:], pg[:], AF.Tanh,
                            bias=bgh_sb[:, j : j + 1], scale=0.5,
                        )
                        pv = pvp.tile([128, TB], F32)
                        for k in range(NCH):
                            nc.tensor.matmul(
                                pv[:],
                                wv_sb[:, k, j * 128 : (j + 1) * 128],
                                yTt[:, k, :],
                                start=(k == 0),
                                stop=(k == NCH - 1),
                            )
                        u = up.tile([128, TB], BF16)
                        nc.vector.scalar_tensor_tensor(
                            u[:], pv[:], bvp_sb[:, j : j + 1], h[:],
                            op0=ALU.add, op1=ALU.mult,
                            accum_out=psums_sb[:, b * NCH + j : b * NCH + j + 1],
                        )
                        # local cumsum (carry starts at 0; cross-core carry is
                        # applied later as a rank-1 correction on the Wo psum)
                        cs_t = up.tile([128, TB], BF16)
                        nc.vector.tensor_tensor_scan(
                            cs_t[:], u[:], u[:],
                            initial=lcarry_sb[:, j : j + 1],
                            op0=ALU.add, op1=ALU.bypass,
                        )
                        nc.vector.tensor_add(
                            lcarry_sb[:, j : j + 1], lcarry_sb[:, j : j + 1],
                            psums_sb[:, b * NCH + j : b * NCH + j + 1],
                        )
                        nc.sync.dma_start(u_d[b, j], cs_t[:])

            # ---------------- carry exchange (pairwise) -----------------
            nc.vector.tensor_add(
                colsum_sb[:], psums_sb[:, 0:NCH], psums_sb[:, NCH : 2 * NCH]
            )
            for b in range(2, NB):
                nc.vector.tensor_add(
                    colsum_sb[:], colsum_sb[:],
                    psums_sb[:, b * NCH : (b + 1) * NCH],
                )
            nc.sync.dma_start(cc_in[:], colsum_sb[:])
            nc.gpsimd.collective_compute(
                "AllReduce",
                ALU.add,
                replica_groups=[[0, 1], [2, 3], [4, 5], [6, 7]],
                ins=[cc_in[:]],
                outs=[cc_out[:]],
            )
            nc.sync.dma_start(allred_sb[:], cc_out[:])
            # carry = gate * (pair_sum - own) = gate * other_half_colsum
            nc.vector.tensor_sub(carry_sb[:], allred_sb[:], colsum_sb[:])
            nc.vector.tensor_scalar_mul(carry_sb[:], carry_sb[:], gate_sb[:, 0:1])
            # w_corr[e] = sum_c carry[c] * Wo[e, c]; broadcast to all partitions
            nc.vector.tensor_copy(carry_bf[:], carry_sb[:])
            with tc.tile_pool(name="pw", bufs=2, space="PSUM") as pwp:
                for m in range(2):
                    pw = pwp.tile([1, 512], F32)
                    for k in range(NCH):
                        nc.tensor.matmul(
                            pw[:],
                            carry_bf[:, k : k + 1],
                            wo_sb[:, k, m * 512 : (m + 1) * 512],
                            start=(k == 0),
                            stop=(k == NCH - 1),
                        )
                    nc.scalar.copy(wc_sb[:, m * 512 : (m + 1) * 512], pw[:])
            nc.sync.dma_start(wcorr_d[:].unsqueeze(0), wc_sb[0:1, :])
            nc.sync.dma_start(
                wcb_sb[:], wcorr_d[:].unsqueeze(0).partition_broadcast(128)
            )

            # ------------ Phase B: Wo matmul, output + carry correction --
            with tc.tile_pool(name="uin", bufs=3) as uinp, \
                 tc.tile_pool(name="ob", bufs=4) as obp, \
                 tc.tile_pool(name="xr", bufs=6) as xrp, \
                 tc.tile_pool(name="po", bufs=6, space="PSUM") as pop:

                for b in range(NB):
                    uin = uinp.tile([128, NCH, TB], BF16)
                    nc.scalar.dma_start(
                        uin[:], u_d[b].rearrange("j p t -> p j t")
                    )
                    for i in range(Nb_T):
                        r0 = (b * Nb_T + i) * 128
                        osb = obp.tile([128, D], F32)
                        xr = xrp.tile([128, D], F32)
                        nc.scalar.dma_start(xr[:], xs[r0 : r0 + 128, :])
                        dslc = dec_sb[:, b * Nb_T + i : b * Nb_T + i + 1]
                        for m in range(2):
                            po = pop.tile([128, 512], F32)
                            for k in range(NCH):
                                nc.tensor.matmul(
                                    po[:],
                                    uin[:, k, i * 128 : (i + 1) * 128],
                                    wo_sb[:, k, m * 512 : (m + 1) * 512],
                                    start=(k == 0),
                                    stop=(k == NCH - 1),
                                )
                            nc.vector.scalar_tensor_tensor(
                                osb[:, m * 512 : (m + 1) * 512], po[:],
                                dslc,
                                xr[:, m * 512 : (m + 1) * 512],
                                op0=ALU.mult, op1=ALU.add,
                            )
                            # + decay * w_corr (cross-core cumsum carry)
                            nc.vector.scalar_tensor_tensor(
                                osb[:, m * 512 : (m + 1) * 512],
                                wcb_sb[:, m * 512 : (m + 1) * 512],
                                dslc,
                                osb[:, m * 512 : (m + 1) * 512],
                                op0=ALU.mult, op1=ALU.add,
                            )
                        if bo_nonzero:
                            nc.vector.tensor_add(osb[:], osb[:], bo_sb[:])
                        nc.sync.dma_start(out[r0 : r0 + 128, :], osb[:])

    return nc


def _prepare_in_maps(x, ln_gamma, ln_beta, Wg, bg, Wv, bv, Wo, bo, log_decay):
    f = np.float32
    x = np.asarray(x, f)
    gamma = np.asarray(ln_gamma, f)
    beta = np.asarray(ln_beta, f)
    Wg = np.asarray(Wg, f)
    Wv = np.asarray(Wv, f)
    Wo = np.asarray(Wo, f)
    bg = np.asarray(bg, f)
    bv = np.asarray(bv, f)
    bo = np.asarray(bo, f)
    log_decay = np.asarray(log_decay, f)

    alpha = np.log1p(np.exp(log_decay)).astype(f)  # softplus
    decay = np.exp(-alpha * np.arange(T, dtype=f)).astype(f)

    bf = ml_dtypes.bfloat16
    def warr(w):  # [d, e] -> [p, k, e] so the SBUF load is contiguous
        return np.ascontiguousarray(
            w.reshape(NCH, 128, D).transpose(1, 0, 2)
        ).astype(bf)
    wgT = warr((Wg * gamma[None, :]).T)
    wvT = warr((Wv * gamma[None, :]).T)
    woT = warr(Wo.T)
    bgh = ((bg + Wg @ beta) / 2).astype(f)
    bvp = (bv + Wv @ beta).astype(f)
    ident = np.eye(128, dtype=ml_dtypes.bfloat16)
    bo_nonzero = bool(np.any(bo != 0))

    in_maps = []
    for c in range(N_CORES):
        b, h = divmod(c, 2)
        m = {
            "xs": np.ascontiguousarray(x[b, h * TL : (h + 1) * TL]),
            "wgT": wgT,
            "wvT": wvT,
            "woT": woT,
            "bgh": bgh,
            "bvp": bvp,
            "dec": np.ascontiguousarray(decay[h * TL : (h + 1) * TL]),
            "gate": np.full((128, 1), float(h), f),
            "ident": ident,
        }
        if bo_nonzero:
            m["bob"] = bo
        in_maps.append(m)
    return in_maps, bo_nonzero


def kernel_with_results(trace=False, **inputs):
    in_maps, bo_nonzero = _prepare_in_maps(**inputs)
    nc = _PROGRAM_CACHE.get(bo_nonzero)
    if nc is None:
        nc = _build_program(bo_nonzero)
        _split_excess_waits(nc)
        _PROGRAM_CACHE[bo_nonzero] = nc
    res = run_bass_kernel_spmd(
        nc, in_maps, list(range(N_CORES)), trace=trace, trace_cores=[0] if trace else None
    )
    out = np.empty((B, T, D), np.float32)
    for c in range(N_CORES):
        b, h = divmod(c, 2)
        out[b, h * TL : (h + 1) * TL] = res.results[c]["out"]
    return out, res


def kernel(**inputs):
    out, _ = kernel_with_results(trace=False, **inputs)
    return out


# revision 16
# speedup vs baseline: 1.1402x; 1.1402x over previous
"""Trainium2 Bass kernel for nn_CRELayer (LayerNorm -> gated push/pull cumsum -> decayed output proj -> residual).

Contract: kernel(**inputs) takes the FULL unsharded inputs (numpy) and returns
the FULL (4, 8192, 1024) float32 output.

Sharding: 8 cores; core c handles (batch b=c//2, T-half h=c%2) -> a (4096, 1024)
row-slice. The time-cumsum crosses the half boundary, so each core computes
per-channel column sums of u = tanh(z/2)*v and a pairwise in-kernel AllReduce
hands the half-0 totals to the half-1 core as the scan's initial state
(gated to zero on half-0 cores).

Math notes:
  - LayerNorm gamma/beta are folded into Wg/Wv + biases on the host.
  - cumsum(g*v) - cumsum((1-g)*v) = cumsum((2g-1)*v) and 2*sigmoid(z)-1 = tanh(z/2),
    so the gate+value combine is a single ACT tanh pass.
  - positional decay is applied per-partition AFTER the Wo matmul:
    (cumsum*decay) @ Wo.T == decay[t] * (cumsum @ Wo.T).
"""

import sys
import types

for _p in ("/opt/trn_rl_repo",):
    if _p not in sys.path:
        sys.path.insert(0, _p)

import ml_dtypes
import numpy as np

# ---------------------------------------------------------------------------
# Environment shims (axon image quirks), applied once at import.
# ---------------------------------------------------------------------------


def _install_axon_hooks_shim():
    """The image's antenv package lacks axon_hooks; provide it so
    run_bass_kernel_spmd(trace=True) can collect NTFF profiles."""
    if "antenv.axon_hooks" in sys.modules:
        return
    hooks = types.ModuleType("antenv.axon_hooks")
    hooks._hook = None

    def set_axon_ntff_profile_hook(h):
        hooks._hook = h

    def get_axon_ntff_profile_hook():
        return hooks._hook

    hooks.set_axon_ntff_profile_hook = set_axon_ntff_profile_hook
    hooks.get_axon_ntff_profile_hook = get_axon_ntff_profile_hook
    sys.modules["antenv.axon_hooks"] = hooks
    try:
        from trn_agent_boot.trn_boot import _ntff_profile_via_ctypes

        set_axon_ntff_profile_hook(
            _ntff_profile_via_ctypes("/opt/axon/libaxon_pjrt.so")
        )
    except Exception:
        pass


_install_axon_hooks_shim()

import bass_rust
import concourse.bass as bass
import concourse.tile as tile
from concourse import mybir
from concourse.bass_utils import run_bass_kernel_spmd
from concourse.vector_clock import ScopedClock


def _patch_tile_drain():
    """This walrus build only accepts one sem wait on sync-engine
    instructions; the stock TileContext exit drain carries one wait per live
    proc. Re-emit those waits as individual sync nops (same semantics)."""
    if getattr(tile.TileContext, "_drain_patch_installed", False):
        return

    def _drain_and_barrier(self, tick_clock, wait_clock):
        nc = self.nc
        probe = nc.sync.nop()
        wait_clock.add_sem_waits(
            probe.ins, ScopedClock({None: tick_clock.global_clock})
        )
        si = probe.ins.sync_info
        waits = list(si.on_wait) if si is not None and si.on_wait else []
        if len(waits) > 1:
            si.on_wait = waits[:1]
            for i in range(1, len(waits)):
                nop2 = nc.sync.nop()
                nop2.ins.sync_info = bass_rust.SyncInfo(
                    on_wait=waits[i : i + 1], on_update=[]
                )
        nc.sync.drain()
        nc.all_engine_barrier()
        assert self.sems is not None
        popped = nc._tile_sem_poison_stack.pop()
        assert popped is self._sem_poison
        nc.clear_and_free_semaphores(list(self.sems.allocated().values()))
        nc.all_engine_barrier()

    tile.TileContext._drain_and_barrier = _drain_and_barrier
    tile.TileContext._drain_patch_installed = True


_patch_tile_drain()

# ---------------------------------------------------------------------------
# Problem constants (hardcoded per contract).
# ---------------------------------------------------------------------------

B, T, D = 4, 8192, 1024
EPS = 1e-5
N_CORES = 8
TL = T // 2          # rows per core
NB = 8               # t-blocks per core
TB = TL // NB        # 512 rows per block
Nb_T = TB // 128     # 4 t-tiles per block
NCH = D // 128       # 8 chunks of 128 channels

F32 = mybir.dt.float32
F32R = mybir.dt.float32r
BF16 = mybir.dt.bfloat16
U32 = mybir.dt.uint32
AF = mybir.ActivationFunctionType
ALU = mybir.AluOpType

_PROGRAM_CACHE = {}

_WAIT_CAP = 1  # this walrus rejects TPB instructions with >1 sync wait


def _split_excess_waits(nc, cap=_WAIT_CAP):
    """Hoist excess sem waits onto same-engine nops placed just before the
    owning instruction. Engine queues execute in order, so a preceding nop
    carrying a wait gates the instruction identically."""
    n_split = 0
    for bb in nc.m.functions[0].blocks:
        insts = bb.instructions
        new_insts = []
        for inst in insts:
            si = inst.sync_info
            waits = list(si.on_wait) if si is not None and si.on_wait else []
            if len(waits) > cap:
                extra = waits[:-cap]
                for w in extra:
                    nop = mybir.InstNoOp(
                        name=f"waitsplit-{n_split}", ins=[], outs=[]
                    )
                    n_split += 1
                    nop.engine = inst.engine
                    nop.sync_info = bass_rust.SyncInfo(
                        on_wait=[w], on_update=[]
                    )
                    new_insts.append(nop)
                si.on_wait = waits[-cap:]
            new_insts.append(inst)
        if n_split:
            bb.instructions = new_insts
    return n_split


def _build_program(bo_nonzero: bool) -> bass.Bass:
    nc = bass.Bass(trn_type="TRN2")

    xs = nc.declare_dram_parameter("xs", [TL, D], F32, isOutput=False)
    wgT = nc.declare_dram_parameter("wgT", [128, NCH, D], BF16, isOutput=False)
    wvT = nc.declare_dram_parameter("wvT", [128, NCH, D], BF16, isOutput=False)
    woT = nc.declare_dram_parameter("woT", [128, NCH, D], BF16, isOutput=False)
    bgh = nc.declare_dram_parameter("bgh", [128, NCH], F32, isOutput=False)
    bvp = nc.declare_dram_parameter("bvp", [128, NCH], F32, isOutput=False)
    dec = nc.declare_dram_parameter("dec", [128, TL // 128], F32, isOutput=False)
    gate = nc.declare_dram_parameter("gate", [128, 1], F32, isOutput=False)
    ident = nc.declare_dram_parameter("ident", [128, 128], BF16, isOutput=False)
    if bo_nonzero:
        bob = nc.declare_dram_parameter("bob", [D], F32, isOutput=False)
    out = nc.declare_dram_parameter("out", [TL, D], F32, isOutput=True)

    u_d = nc.dram_tensor("u_spill", [NB, NCH, 128, TB], BF16)
    cc_in = nc.dram_tensor("cc_in", [128, NCH], F32)
    cc_out = nc.dram_tensor("cc_out", [128, NCH], F32)
    wcorr_d = nc.dram_tensor("wcorr", [D], F32)

    with tile.TileContext(nc) as tc:
        with tc.tile_pool(name="const", bufs=1) as cpool:
            ident_sb = cpool.tile([128, 128], BF16)
            nc.sync.dma_start(ident_sb[:], ident[:])
            bgh_sb = cpool.tile([128, NCH], F32)
            nc.sync.dma_start(bgh_sb[:], bgh[:])
            bvp_sb = cpool.tile([128, NCH], F32)
            nc.sync.dma_start(bvp_sb[:], bvp[:])
            dec_sb = cpool.tile([128, TL // 128], F32)
            nc.sync.dma_start(dec_sb[:], dec[:])
            gate_sb = cpool.tile([128, 1], F32)
            nc.sync.dma_start(gate_sb[:], gate[:])
            if bo_nonzero:
                bo_sb = cpool.tile([128, D], F32)
                nc.sync.dma_start(
                    bo_sb[:], bob[:].unsqueeze(0).partition_broadcast(128)
                )
            magic_sb = cpool.tile([128, Nb_T], U32)
            nc.vector.memset(magic_sb[:], 0x5F3759DF)
            psums_sb = cpool.tile([128, NB * NCH], F32)
            carry_sb = cpool.tile([128, NCH], F32)
            lcarry_sb = cpool.tile([128, NCH], F32)
            nc.vector.memset(lcarry_sb[:], 0.0)
            carry_bf = cpool.tile([128, NCH], BF16)
            wc_sb = cpool.tile([1, D], F32)
            wcb_sb = cpool.tile([128, D], F32)
            wo_sb = cpool.tile([128, NCH, D], BF16)
            nc.scalar.dma_start(wo_sb[:], woT[:])
            colsum_sb = cpool.tile([128, NCH], F32)
            allred_sb = cpool.tile([128, NCH], F32)

            # Phase-B SBUF pools are allocated FIRST so they do not alias
            # phase-A pool memory (stack allocator overlap-deps would
            # otherwise serialize phase B behind the full phase-A drain).
            pbstack = [
                tc.tile_pool(name="uin", bufs=3),
                tc.tile_pool(name="ob", bufs=3),
                tc.tile_pool(name="xr", bufs=4),
            ]
            uinp = pbstack[0].__enter__()
            obp = pbstack[1].__enter__()
            xrp = pbstack[2].__enter__()

            # ---------------- Phase A: LN -> transpose -> Wg/Wv -> u ----
            with tc.tile_pool(name="wA", bufs=1) as wpool, \
                 tc.tile_pool(name="xt", bufs=6) as xtp, \
                 tc.tile_pool(name="xb", bufs=6) as xbp, \
                 tc.tile_pool(name="yT", bufs=3) as yTp, \
                 tc.tile_pool(name="hv", bufs=6) as hvp, \
                 tc.tile_pool(name="ub", bufs=6) as up, \
                 tc.tile_pool(name="stat", bufs=10) as stp, \
                 tc.tile_pool(name="ptr", bufs=2, space="PSUM") as ptrp, \
                 tc.tile_pool(name="pg", bufs=3, space="PSUM") as pgp, \
                 tc.tile_pool(name="pv", bufs=3, space="PSUM") as pvp:

                wg_sb = wpool.tile([128, NCH, D], BF16)
                nc.scalar.dma_start(wg_sb[:], wgT[:])
                wv_sb = wpool.tile([128, NCH, D], BF16)
                nc.scalar.dma_start(wv_sb[:], wvT[:])

                for b in range(NB):
                    xts = []
                    mvs = stp.tile([128, Nb_T, 2], F32)
                    raw = []
                    for i in range(Nb_T):
                        r0 = (b * Nb_T + i) * 128
                        xt = xtp.tile([128, D], F32)
                        nc.sync.dma_start(xt[:], xs[r0 : r0 + 128, :])
                        st6 = stp.tile([128, 2, 6], F32)
                        for a in range(2):
                            nc.vector.bn_stats(
                                st6[:, a, :], xt[:, a * 512 : (a + 1) * 512]
                            )
                        nc.vector.bn_aggr(mvs[:, i, :], st6[:])
                        raw.append(xt)
                    # s = rsqrt(var+eps) for the whole block: magic + 3 Newton
                    w = stp.tile([128, Nb_T], F32)
                    nc.vector.tensor_scalar_add(w[:], mvs[:, :, 1], EPS)
                    ti = stp.tile([128, Nb_T], U32)
                    nc.vector.tensor_scalar(
                        ti[:], w[:].bitcast(U32), 1, None,
                        op0=ALU.logical_shift_right,
                    )
                    nc.vector.tensor_sub(ti[:], magic_sb[:], ti[:])
                    sv = ti[:].bitcast(F32)
                    t2 = stp.tile([128, Nb_T], F32)
                    for _ in range(2):
                        nc.vector.tensor_mul(t2[:], sv, sv)
                        nc.vector.tensor_mul(t2[:], t2[:], w[:])
                        nc.vector.tensor_scalar(
                            t2[:], t2[:], -0.5, 1.5, op0=ALU.mult, op1=ALU.add
                        )
                        nc.vector.tensor_mul(sv, sv, t2[:])
                    for i in range(Nb_T):
                        xb = xbp.tile([128, D], BF16)
                        nc.vector.tensor_scalar(
                            xb[:], raw[i][:], mvs[:, i, 0:1], sv[:, i : i + 1],
                            op0=ALU.subtract, op1=ALU.mult,
                        )
                        xts.append(xb)

                    yTt = yTp.tile([128, NCH, TB], BF16)
                    for k in range(NCH):
                        ptr = ptrp.tile([128, TB], BF16)
                        for i in range(Nb_T):
                            nc.tensor.transpose(
                                ptr[:, i * 128 : (i + 1) * 128],
                                xts[i][:, k * 128 : (k + 1) * 128],
                                ident_sb[:],
                            )
                        nc.scalar.copy(yTt[:, k, :], ptr[:])

                    for j in range(NCH):
                        pg = pgp.tile([128, TB], F32)
                        for k in range(NCH):
                            nc.tensor.matmul(
                                pg[:],
                                wg_sb[:, k, j * 128 : (j + 1) * 128],
                                yTt[:, k, :],
                                start=(k == 0),
                                stop=(k == NCH - 1),
                            )
                        h = hvp.tile([128, TB], BF16)
                        nc.scalar.activation(
                            h[:], pg[:], AF.Tanh,
                            bias=bgh_sb[:, j : j + 1], scale=0.5,
                        )
                        pv = pvp.tile([128, TB], F32)
                        for k in range(NCH):
                            nc.tensor.matmul(
                                pv[:],
                                wv_sb[:, k, j * 128 : (j + 1) * 128],
                                yTt[:, k, :],
                                start=(k == 0),
                                stop=(k == NCH - 1),
                            )
                        u = up.tile([128, TB], BF16)
                        nc.vector.scalar_tensor_tensor(
                            u[:], pv[:], bvp_sb[:, j : j + 1], h[:],
                            op0=ALU.add, op1=ALU.mult,
                            accum_out=psums_sb[:, b * NCH + j : b * NCH + j + 1],
                        )
                        # local cumsum (carry starts at 0; cross-core carry is
                        # applied later as a rank-1 correction on the Wo psum)
                        cs_t = up.tile([128, TB], BF16)
                        nc.vector.tensor_tensor_scan(
                            cs_t[:], u[:], u[:],
                            initial=lcarry_sb[:, j : j + 1],
                            op0=ALU.add, op1=ALU.bypass,
                        )
                        nc.vector.tensor_add(
                            lcarry_sb[:, j : j + 1], lcarry_sb[:, j : j + 1],
                            psums_sb[:, b * NCH + j : b * NCH + j + 1],
                        )
                        nc.sync.dma_start(u_d[b, j], cs_t[:])

            # ---------------- carry exchange (pairwise) -----------------
            nc.vector.tensor_add(
                colsum_sb[:], psums_sb[:, 0:NCH], psums_sb[:, NCH : 2 * NCH]
            )
            for b in range(2, NB):
                nc.vector.tensor_add(
                    colsum_sb[:], colsum_sb[:],
                    psums_sb[:, b * NCH : (b + 1) * NCH],
                )
            nc.sync.dma_start(cc_in[:], colsum_sb[:])
            nc.gpsimd.collective_compute(
                "AllReduce",
                ALU.add,
                replica_groups=[[0, 1], [2, 3], [4, 5], [6, 7]],
                ins=[cc_in[:]],
                outs=[cc_out[:]],
            )
            nc.sync.dma_start(allred_sb[:], cc_out[:])
            # carry = gate * (pair_sum - own) = gate * other_half_colsum
            nc.vector.tensor_sub(carry_sb[:], allred_sb[:], colsum_sb[:])
            nc.vector.tensor_scalar_mul(carry_sb[:], carry_sb[:], gate_sb[:, 0:1])
            # w_corr[e] = sum_c carry[c] * Wo[e, c]; broadcast to all partitions
            nc.vector.tensor_copy(carry_bf[:], carry_sb[:])
            with tc.tile_pool(name="pw", bufs=2, space="PSUM") as pwp:
                for m in range(2):
                    pw = pwp.tile([1, 512], F32)
                    for k in range(NCH):
                        nc.tensor.matmul(
                            pw[:],
                            carry_bf[:, k : k + 1],
                            wo_sb[:, k, m * 512 : (m + 1) * 512],
                            start=(k == 0),
                            stop=(k == NCH - 1),
                        )
                    nc.vector.tensor_copy(wc_sb[:, m * 512 : (m + 1) * 512], pw[:])
            nc.sync.dma_start(wcorr_d[:].unsqueeze(0), wc_sb[0:1, :])
            nc.sync.dma_start(
                wcb_sb[:], wcorr_d[:].unsqueeze(0).partition_broadcast(128)
            )

            # ------------ Phase B: Wo matmul, output + carry correction --
            with tc.tile_pool(name="po", bufs=6, space="PSUM") as pop:

                for b in range(NB):
                    uin = uinp.tile([128, NCH, TB], BF16)
                    nc.gpsimd.dma_start(
                        uin[:], u_d[b].rearrange("j p t -> p j t")
                    )
                    for i in range(Nb_T):
                        r0 = (b * Nb_T + i) * 128
                        osb = obp.tile([128, D], F32)
                        xr = xrp.tile([128, D], F32)
                        nc.gpsimd.dma_start(xr[:], xs[r0 : r0 + 128, :])
                        dslc = dec_sb[:, b * Nb_T + i : b * Nb_T + i + 1]
                        for m in range(2):
                            po = pop.tile([128, 512], F32)
                            for k in range(NCH):
                                nc.tensor.matmul(
                                    po[:],
                                    uin[:, k, i * 128 : (i + 1) * 128],
                                    wo_sb[:, k, m * 512 : (m + 1) * 512],
                                    start=(k == 0),
                                    stop=(k == NCH - 1),
                                )
                            nc.vector.scalar_tensor_tensor(
                                osb[:, m * 512 : (m + 1) * 512], po[:],
                                dslc,
                                xr[:, m * 512 : (m + 1) * 512],
                                op0=ALU.mult, op1=ALU.add,
                            )
                            # + decay * w_corr (cross-core cumsum carry)
                            nc.vector.scalar_tensor_tensor(
                                osb[:, m * 512 : (m + 1) * 512],
                                wcb_sb[:, m * 512 : (m + 1) * 512],
                                dslc,
                                osb[:, m * 512 : (m + 1) * 512],
                                op0=ALU.mult, op1=ALU.add,
                            )
                        if bo_nonzero:
                            nc.vector.tensor_add(osb[:], osb[:], bo_sb[:])
                        nc.sync.dma_start(out[r0 : r0 + 128, :], osb[:])

            for p in reversed(pbstack):
                p.__exit__(None, None, None)

    return nc


def _prepare_in_maps(x, ln_gamma, ln_beta, Wg, bg, Wv, bv, Wo, bo, log_decay):
    f = np.float32
    x = np.asarray(x, f)
    gamma = np.asarray(ln_gamma, f)
    beta = np.asarray(ln_beta, f)
    Wg = np.asarray(Wg, f)
    Wv = np.asarray(Wv, f)
    Wo = np.asarray(Wo, f)
    bg = np.asarray(bg, f)
    bv = np.asarray(bv, f)
    bo = np.asarray(bo, f)
    log_decay = np.asarray(log_decay, f)

    alpha = np.log1p(np.exp(log_decay)).astype(f)  # softplus
    decay = np.exp(-alpha * np.arange(T, dtype=f)).astype(f)

    bf = ml_dtypes.bfloat16
    def warr(w):  # [d, e] -> [p, k, e] so the SBUF load is contiguous
        return np.ascontiguousarray(
            w.reshape(NCH, 128, D).transpose(1, 0, 2)
        ).astype(bf)
    wgT = warr((Wg * gamma[None, :]).T)
    wvT = warr((Wv * gamma[None, :]).T)
    woT = warr(Wo.T)
    bgh = np.ascontiguousarray(((bg + Wg @ beta) / 2).reshape(NCH, 128).T).astype(f)
    bvp = np.ascontiguousarray((bv + Wv @ beta).reshape(NCH, 128).T).astype(f)
    ident = np.eye(128, dtype=ml_dtypes.bfloat16)
    bo_nonzero = bool(np.any(bo != 0))

    in_maps = []
    for c in range(N_CORES):
        b, h = divmod(c, 2)
        m = {
            "xs": np.ascontiguousarray(x[b, h * TL : (h + 1) * TL]),
            "wgT": wgT,
            "wvT": wvT,
            "woT": woT,
            "bgh": bgh,
            "bvp": bvp,
            "dec": np.ascontiguousarray(
                decay[h * TL : (h + 1) * TL].reshape(TL // 128, 128).T
            ),
            "gate": np.full((128, 1), float(h), f),
            "ident": ident,
        }
        if bo_nonzero:
            m["bob"] = bo
        in_maps.append(m)
    return in_maps, bo_nonzero


def kernel_with_results(trace=False, **inputs):
    in_maps, bo_nonzero = _prepare_in_maps(**inputs)
    nc = _PROGRAM_CACHE.get(bo_nonzero)
    if nc is None:
        nc = _build_program(bo_nonzero)
        _split_excess_waits(nc)
        _PROGRAM_CACHE[bo_nonzero] = nc
    res = run_bass_kernel_spmd(
        nc, in_maps, list(range(N_CORES)), trace=trace, trace_cores=[0] if trace else None
    )
    out = np.empty((B, T, D), np.float32)
    for c in range(N_CORES):
        b, h = divmod(c, 2)
        out[b, h * TL : (h + 1) * TL] = res.results[c]["out"]
    return out, res


def kernel(**inputs):
    out, _ = kernel_with_results(trace=False, **inputs)
    return out
